# revision 2
# baseline (speedup 1.0000x reference)
"""HashGrid1D forward on 8 trn2 NeuronCores.

Strategy: the 12 resolutions are all powers of two (16..32768), so the whole
module is piecewise-linear in x with nodes at u/32768.  Precompute per-segment
value/slope rows AB[u] = [V(u) | V(u+1)-V(u)] (24+24 f32).  On the host,
bucket samples by segment u (a layout/sharding permutation); each core gets a
contiguous range of 4096 segments with its samples padded to a fixed S slots
per segment.  On device, partition p of tile T owns segment T*128+p, so the
segment's A/B row is a per-partition scalar: out = A + t*B is one fused
tensor_scalar op per channel over [128, S] sample slots.  All DMA is static
and contiguous; t is computed on device from x.
"""

import numpy as np

BATCH = 4_194_304
HASH = 16384
L, F = 12, 2
CH = L * F                      # 24 output channels
NSEG = 32768                    # finest-level segments
N_CORES = 8
SEG_PER_CORE = NSEG // N_CORES  # 4096
TILES = SEG_PER_CORE // 128     # 32 tiles of 128 segments


def _build_ab(table: np.ndarray) -> np.ndarray:
    """AB[u] = [V(u) (24) | V(u+1)-V(u) (24)] in f32, V = node values (f64 math)."""
    tab = table.reshape(HASH, L, F).astype(np.float64)
    u = np.arange(NSEG + 1)                       # nodes 0..32768
    V = np.empty((NSEG + 1, L, F), np.float64)
    for l in range(L):
        s = 11 - l                                # repeat shift for this level
        i0 = u >> s
        w = ((u & ((1 << s) - 1)) / (1 << s))[:, None]
        e0 = tab[i0 & (HASH - 1), l]              # [NSEG+1, F]
        e1 = tab[(i0 + 1) & (HASH - 1), l]
        V[:, l] = (1.0 - w) * e0 + w * e1
    V = V.reshape(NSEG + 1, CH)
    ab = np.empty((NSEG, 2 * CH), np.float32)
    ab[:, :CH] = V[:-1].astype(np.float32)
    ab[:, CH:] = (V[1:] - V[:-1]).astype(np.float32)
    return ab


def _build_nc(S: int, reps: int = 1):
    import concourse.bass as bass
    import concourse.mybir as mybir
    import concourse.tile as tile
    from concourse import bacc
    from contextlib import ExitStack

    DT = mybir.dt.float32
    nc = bacc.Bacc("TRN2", target_bir_lowering=False)
    xb = nc.dram_tensor("xb", [SEG_PER_CORE * S], DT, kind="ExternalInput")
    ab = nc.dram_tensor("ab", [SEG_PER_CORE, 2 * CH], DT, kind="ExternalInput")
    outp = nc.dram_tensor("outp", [SEG_PER_CORE * S, CH], DT, kind="ExternalOutput")

    with tile.TileContext(nc) as tc, ExitStack() as ctx:
        const = ctx.enter_context(tc.tile_pool(name="const", bufs=1))
        work = ctx.enter_context(tc.tile_pool(name="work", bufs=2))

        # resident x and AB for the whole core slice
        xt_all = const.tile([128, TILES, S], DT)
        nc.gpsimd.dma_start(
            out=xt_all[:],
            in_=xb[:].rearrange("(T p s) -> p T s", T=TILES, p=128, s=S),
        )
        ab_all = const.tile([128, TILES, 2 * CH], DT)
        nc.gpsimd.dma_start(
            out=ab_all[:],
            in_=ab[:].rearrange("(T p) c -> p T c", T=TILES, p=128),
        )
        # ucol[p, T] = T*128 + p  (local segment index of partition p in tile T)
        ucol_i = const.tile([128, TILES], mybir.dt.int32)
        nc.gpsimd.iota(ucol_i[:], pattern=[[128, TILES]], base=0, channel_multiplier=1)
        ucol_f = const.tile([128, TILES], DT)
        nc.vector.tensor_copy(ucol_f[:], ucol_i[:])

        out_view = outp[:].rearrange("(T p s) c -> T p s c", T=TILES, p=128, s=S)
        for _ in range(reps):
            for T in range(TILES):
                t_t = work.tile([128, S], DT, tag="t")
                # t = x*32768 - u_local   (exact in f32)
                nc.vector.tensor_scalar(
                    out=t_t[:], in0=xt_all[:, T, :],
                    scalar1=32768.0, scalar2=ucol_f[:, T : T + 1],
                    op0=mybir.AluOpType.mult, op1=mybir.AluOpType.subtract,
                )
                o_t = work.tile([128, S, CH], DT, tag="o")
                for c in range(CH):
                    # out[:, :, c] = t*B[c] + A[c]
                    nc.vector.tensor_scalar(
                        out=o_t[:, :, c], in0=t_t[:],
                        scalar1=ab_all[:, T, CH + c : CH + c + 1],
                        scalar2=ab_all[:, T, c : c + 1],
                        op0=mybir.AluOpType.mult, op1=mybir.AluOpType.add,
                    )
                nc.gpsimd.dma_start(out=out_view[T], in_=o_t[:])
    nc.finalize()
    return nc


def _prep(x: np.ndarray, table: np.ndarray, S=None):
    """Host-side layout: bucket samples by segment, pad to S slots/segment."""
    x = np.clip(x.astype(np.float32), 0.0, 1.0)
    pos = x * np.float32(32768.0)                  # exact (power of two)
    uf = np.floor(pos)
    u = np.minimum(uf.astype(np.int64), NSEG - 1)
    counts = np.bincount(u, minlength=NSEG)
    if S is None:
        S = max(64, int(np.ceil(counts.max() / 32) * 32))
    order = np.argsort(u, kind="stable")
    starts = np.zeros(NSEG, np.int64)
    np.cumsum(counts[:-1], out=starts[1:])
    rank = np.arange(BATCH, dtype=np.int64) - starts[u[order]]
    slot = u[order] * S + rank                     # padded position per sample
    # padded, core-shifted x:  xs = x - c/8 (exact); pad -> t = 0
    useg = np.arange(NSEG, dtype=np.int64)
    xs_pad = ((useg % SEG_PER_CORE).astype(np.float32) / np.float32(32768.0))
    xs_pad = np.repeat(xs_pad, S)
    core_of = slot // (SEG_PER_CORE * S)
    xs_pad[slot] = x[order] - (core_of.astype(np.float32) / np.float32(8.0))
    inv = np.empty(BATCH, np.int64)
    inv[order] = slot                              # out_full[i] = out_pad[inv[i]]
    return xs_pad, inv, S


_cache = {}


def kernel(x: np.ndarray, table: np.ndarray, _reps: int = 1) -> np.ndarray:
    from concourse.bass_utils import run_bass_kernel_spmd

    xs_pad, inv, S = _prep(x, table)
    ab = _build_ab(table.astype(np.float32))
    key = (S, _reps)
    if key not in _cache:
        _cache[key] = _build_nc(S, _reps)
    nc = _cache[key]

    per = SEG_PER_CORE * S
    in_maps = [
        {"xb": xs_pad[c * per : (c + 1) * per],
         "ab": ab[c * SEG_PER_CORE : (c + 1) * SEG_PER_CORE]}
        for c in range(N_CORES)
    ]
    res = run_bass_kernel_spmd(nc, in_maps, list(range(N_CORES)))
    out_pad = np.concatenate([res.results[c]["outp"] for c in range(N_CORES)], axis=0)
    return out_pad[inv]
